# revision 25
# baseline (speedup 1.0000x reference)
"""CGCNN (crystal-graph conv) forward pass on 8 Trainium2 NeuronCores.

Strategy (graph partitioned by destination node, SPMD across 8 cores):
  - Host: sort edges by dst, bucket them into 128-node "windows", pad each
    window's edge list to T*128 slots, assign contiguous window ranges to
    cores.  All per-edge streams are laid out host-side in the transposed
    [128, chunk] order the device wants.
  - Per CGConv layer the z @ W matmul is factored:
        z @ W = h[dst] @ W_dst + h[src] @ W_src + ea1 @ W_e
    Node projections (f||s gates concatenated, 184 wide) are stored in
    bf16 row tables padded to 256 elements (512B rows -> full-rate DMA
    descriptors).  The per-edge "pre-activation" tile is assembled as:
        PE:  pre  = eaT1_chunk.T @ W_e           (inline edge-attr proj)
        PE:  pre += I128 @ G                     (merge of gathered rows)
    where G = P_src[src] (indirect-DMA gather) + P_dst[dst] (indirect-DMA
    gather with CCE accumulate) is built entirely by the DMA engines.
  - gate = sigmoid(pre_f), sp = softplus(pre_s) on ScalarE (batched APs),
    msg = gate*sp on VectorE, segment-sum via one-hot matmul on PE
    (S_T built by a single tensor_scalar is_equal against an iota tile),
    accumulated in PSUM per 128-node window.
  - BatchNorm statistics: per-window ones-matmul partial sums + AllReduce.
  - Node state h stays resident in SBUF per core; projections for the
    next layer are computed locally and the src table is AllGathered.
  - Final MLP + scatter-mean pooling: per-core partial graph sums, summed
    on the host (along with /count and +b2).
"""

import math
import os
import sys

import numpy as np

TRN_REPO = "/opt/trn_rl_repo"

N_NODES = 50000
N_EDGES = 1600000
ATOM_DIM = 92
EDGE_DIM = 41
N_LAYERS = 4
N_GRAPHS = 256
BN_EPS = 1e-5
NEG_SLOPE = 0.01
N_CORES = 8

EA_ROWS = EDGE_DIM + 1  # edge_attr rows + ones row (bias folded in)
PREW = 2 * ATOM_DIM  # 184: [f-gate | s-gate] pre-activation width
TBL = 256  # table row length in bf16 elements (512B rows)
PW = 2 * TBL  # node-projection psum width (dst 0:184, src 256:440)


SKIP_COLL = False  # timing-ablation flag (wrong results when True)


def _ensure_path():
    if TRN_REPO not in sys.path:
        sys.path.insert(0, TRN_REPO)


class Cfg:
    def __init__(self, n_cores, W, T, n_real_nodes, n_graphs, n_layers=N_LAYERS,
                 fp32=False):
        self.fp32 = fp32
        self.n_cores = n_cores
        self.W = W  # windows (=node chunks of 128) per core
        self.T = T  # edge chunks of 128 per window
        self.NPC = W * 128  # padded nodes per core
        self.N_pad = self.NPC * n_cores
        self.n_real_nodes = n_real_nodes
        self.n_graphs = n_graphs
        self.n_layers = n_layers
        # edge-chunk groups per window (PSUM pre-tile batch size)
        g = 6
        self.groups = [g] * (T // g) + ([T % g] if T % g else [])


# --------------------------------------------------------------------------
# host-side preprocessing
# --------------------------------------------------------------------------

def preprocess(inputs, n_cores=N_CORES, fp32=False):
    from ml_dtypes import bfloat16

    x = np.asarray(inputs["x"], np.float32)
    ei = np.asarray(inputs["edge_index"])
    ea = np.asarray(inputs["edge_attr"], np.float32)
    batch = np.asarray(inputs["batch"]).astype(np.int64)
    Wf = np.asarray(inputs["Wf"], np.float32)
    bf = np.asarray(inputs["bf"], np.float32)
    Ws = np.asarray(inputs["Ws"], np.float32)
    bs = np.asarray(inputs["bs"], np.float32)
    gamma = np.asarray(inputs["gamma"], np.float32)
    beta = np.asarray(inputs["beta"], np.float32)
    W1 = np.asarray(inputs["W1"], np.float32)
    b1 = np.asarray(inputs["b1"], np.float32)
    W2 = np.asarray(inputs["W2"], np.float32)
    b2 = np.asarray(inputs["b2"], np.float32)

    n = x.shape[0]
    e = ei.shape[1]
    n_layers = Wf.shape[0]
    n_graphs = N_GRAPHS if n >= 1000 else int(batch.max()) + 1

    src = ei[0].astype(np.int64)
    dst = ei[1].astype(np.int64)

    # node padding / core spans
    W_glob_min = (n + 127) // 128
    W = (W_glob_min + n_cores - 1) // n_cores  # windows per core
    W_glob = W * n_cores
    NPC = W * 128
    N_pad = NPC * n_cores

    # sort edges by dst
    order = np.argsort(dst, kind="stable")
    src_s = src[order]
    dst_s = dst[order]
    ea_s = ea[order]

    win = (dst_s // 128).astype(np.int64)
    counts = np.bincount(win, minlength=W_glob)
    T = max(1, int(math.ceil(counts.max() / 128)))
    spw = T * 128  # slots per window

    # slot position of every edge: window base + rank within window
    cum = np.zeros(W_glob + 1, np.int64)
    np.cumsum(counts, out=cum[1:])
    within = np.arange(e, dtype=np.int64) - cum[win]
    slot = win * spw + within

    n_slots = W_glob * spw
    slot_src = np.zeros(n_slots, np.int32)
    slot_dst_loc = np.zeros(n_slots, np.int32)  # core-local dst node id
    slot_dwin = np.full(n_slots, -1.0, np.float32)  # dst id within window
    slot_ea = np.zeros((n_slots, EDGE_DIM), np.float32)

    slot_src[slot] = src_s.astype(np.int32)
    slot_dst_loc[slot] = (dst_s - (win // W) * NPC).astype(np.int32)
    slot_dwin[slot] = (dst_s % 128).astype(np.float32)
    slot_ea[slot] = ea_s

    cfg = Cfg(n_cores, W, T, n, n_graphs, n_layers, fp32=fp32)
    bff = np.float32 if fp32 else bfloat16

    # per-core, per-window [128, T] layouts (partition = edge-in-chunk)
    def per_core_idx(a, dtype):
        # [W_glob*spw] -> [n_cores, W, T, 128] -> [n_cores, W, 128, T]
        r = a.reshape(n_cores, W, T, 128).transpose(0, 1, 3, 2)
        return np.ascontiguousarray(r).astype(dtype)

    src_idx = per_core_idx(slot_src, np.int32)
    dst_idx = per_core_idx(slot_dst_loc, np.int32)
    dwin = per_core_idx(slot_dwin, bff)

    # eaT1: [n_cores, EA_ROWS, W, spw]  (feature-major, ones row appended)
    ea_t = np.ones((n_slots, EA_ROWS), np.float32)
    ea_t[:, :EDGE_DIM] = slot_ea
    ea_t = ea_t.reshape(n_cores, W, spw, EA_ROWS).transpose(0, 3, 1, 2)
    eaT1 = np.ascontiguousarray(ea_t).astype(bff)

    # node features per core (padded)
    x_pad = np.zeros((N_pad, ATOM_DIM), np.float32)
    x_pad[:n] = x
    x_own = x_pad.reshape(n_cores, NPC, ATOM_DIM).copy()

    # batch one-hot columns: [n_cores, 128, W], sentinel 512 for pad nodes
    bpad = np.full(N_pad, 512.0, np.float32)
    bpad[:n] = batch.astype(np.float32)
    bcols = bpad.reshape(n_cores, W, 128).transpose(0, 2, 1)
    batchA = np.ascontiguousarray(bcols).astype(bff)
    batchB = np.ascontiguousarray(bcols - 128.0).astype(bff)

    # ---- weights ----
    # W_e: [L, EA_ROWS, PREW]  rows = edge_attr features + bias row
    W_e = np.zeros((n_layers, EA_ROWS, PREW), np.float32)
    W_e[:, :EDGE_DIM, :ATOM_DIM] = Wf[:, 2 * ATOM_DIM:, :]
    W_e[:, :EDGE_DIM, ATOM_DIM:] = Ws[:, 2 * ATOM_DIM:, :]
    W_e[:, EDGE_DIM, :ATOM_DIM] = bf
    W_e[:, EDGE_DIM, ATOM_DIM:] = bs
    W_e = W_e.astype(bff)

    # W_n: [L, ATOM_DIM, PW]; cols 0:184 dst proj, 256:440 src proj, rest 0
    W_n = np.zeros((n_layers, ATOM_DIM, PW), np.float32)
    W_n[:, :, 0:ATOM_DIM] = Wf[:, 0:ATOM_DIM, :]
    W_n[:, :, ATOM_DIM:PREW] = Ws[:, 0:ATOM_DIM, :]
    W_n[:, :, TBL:TBL + ATOM_DIM] = Wf[:, ATOM_DIM:2 * ATOM_DIM, :]
    W_n[:, :, TBL + ATOM_DIM:TBL + PREW] = Ws[:, ATOM_DIM:2 * ATOM_DIM, :]
    W_n = W_n.astype(bff)

    gb = np.stack([gamma, beta], axis=1).astype(np.float32)  # [L, 2, A]
    gb = gb.reshape(1, n_layers * 2, ATOM_DIM)

    # consts
    iota = np.broadcast_to(np.arange(128, dtype=np.float32), (128, 128))
    iota_t = np.ascontiguousarray(iota).astype(bff)
    ident_bf = np.eye(128, dtype=np.float32).astype(bff)
    ident_f = np.eye(128, dtype=np.float32)
    ones_col = np.ones((128, 1), np.float32)
    ones_row = np.ones((1, 128), np.float32)

    shared = {
        "eaW": W_e, "nW": W_n, "gb": gb,
        "W1": W1.astype(bff), "b1col": b1.reshape(-1, 1).astype(np.float32),
        "W2": W2.astype(bff),
        "iota_t": iota_t, "ident_bf": ident_bf, "ident_f": ident_f,
        "ones_col": ones_col, "ones_row": ones_row,
    }
    in_maps = []
    for c in range(n_cores):
        m = dict(shared)
        m["x_own"] = x_own[c]
        m["src_idx"] = src_idx[c]
        m["dst_idx"] = dst_idx[c]
        m["dwin"] = dwin[c]
        m["eaT1"] = eaT1[c]
        m["batchA"] = batchA[c]
        m["batchB"] = batchB[c]
        in_maps.append(m)

    cnt = np.bincount(batch, minlength=n_graphs).astype(np.float32)
    post = {"counts": cnt, "b2": float(b2[0])}
    return cfg, in_maps, post


# --------------------------------------------------------------------------
# device program
# --------------------------------------------------------------------------

def build_program(cfg):
    _ensure_path()
    from concourse import bacc, bass, mybir, tile

    dt = mybir.dt
    Alu = mybir.AluOpType
    Act = mybir.ActivationFunctionType
    C, W, T = cfg.n_cores, cfg.W, cfg.T
    NPC, L = cfg.NPC, cfg.n_layers
    A = ATOM_DIM
    NG2 = 2 if cfg.n_graphs > 128 else 1
    rg = [list(range(C))]
    shared_as = "Shared" if C > 4 else "Local"

    bft = dt.float32 if cfg.fp32 else dt.bfloat16
    if cfg.fp32:
        TW = TBL  # table row width in f32 elements

        def rhsG(Gt, c):
            return Gt[:, c, 0:PREW]
    else:
        TW = TBL // 2  # bf16 pairs packed in f32 words

        def rhsG(Gt, c):
            return Gt[:, c, :].bitcast(dt.bfloat16)[:, 0:PREW]
    nc = bacc.Bacc("TRN2", target_bir_lowering=False, debug=False,
                   num_devices=C)

    # ---- I/O ----
    def din(name, shape, d):
        return nc.dram_tensor(name, list(shape), d, kind="ExternalInput")

    x_own = din("x_own", (NPC, A), dt.float32)
    src_idx = din("src_idx", (W, 128, T), dt.int32)
    dst_idx = din("dst_idx", (W, 128, T), dt.int32)
    dwin = din("dwin", (W, 128, T), bft)
    eaT1 = din("eaT1", (EA_ROWS, W, T * 128), bft)
    batchA = din("batchA", (128, W), bft)
    batchB = din("batchB", (128, W), bft)
    eaW = din("eaW", (L, EA_ROWS, PREW), bft)
    nW = din("nW", (L, A, PW), bft)
    gb = din("gb", (1, L * 2, A), dt.float32)
    W1 = din("W1", (A, 128), bft)
    b1col = din("b1col", (128, 1), dt.float32)
    W2 = din("W2", (128, 1), bft)
    iota_in = din("iota_t", (128, 128), bft)
    identb_in = din("ident_bf", (128, 128), bft)
    identf_in = din("ident_f", (128, 128), dt.float32)
    onesc_in = din("ones_col", (128, 1), dt.float32)
    onesr_in = din("ones_row", (1, 128), dt.float32)

    out_part = nc.dram_tensor("out_partial", [128, NG2], dt.float32,
                              kind="ExternalOutput")

    with tile.TileContext(nc) as tc:
        with (
            tc.tile_pool(name="const", bufs=1) as cpool,
            tc.tile_pool(name="state", bufs=1) as spool,
            tc.tile_pool(name="dramt", bufs=2, space="DRAM") as dpool,
        ):
            # ---- persistent SBUF state ----
            h_own = spool.tile([128, W, A], dt.float32)
            agg_own = spool.tile([128, W, A], dt.float32)

            # ---- consts to SBUF ----
            iota_sb = cpool.tile([128, 128], bft)
            nc.sync.dma_start(iota_sb[:], iota_in[:])
            identb_sb = cpool.tile([128, 128], bft)
            nc.sync.dma_start(identb_sb[:], identb_in[:])
            identf_sb = cpool.tile([128, 128], dt.float32)
            nc.sync.dma_start(identf_sb[:], identf_in[:])
            onesc_sb = cpool.tile([128, 1], dt.float32)
            nc.sync.dma_start(onesc_sb[:], onesc_in[:])
            onesr_sb = cpool.tile([1, 128], dt.float32)
            nc.sync.dma_start(onesr_sb[:], onesr_in[:])
            eaW_sb = cpool.tile([EA_ROWS, L, PREW], bft)
            nc.sync.dma_start(eaW_sb[:], eaW[:].rearrange("l r w -> r l w"))
            nW_sb = cpool.tile([A, L, PW], bft)
            nc.sync.dma_start(nW_sb[:], nW[:].rearrange("l a w -> a l w"))
            gb_sb = cpool.tile([1, L * 2, A], dt.float32)
            nc.sync.dma_start(gb_sb[:], gb[:])
            W1_sb = cpool.tile([A, 128], bft)
            nc.sync.dma_start(W1_sb[:], W1[:])
            b1_sb = cpool.tile([128, 1], dt.float32)
            nc.sync.dma_start(b1_sb[:], b1col[:])
            W2_sb = cpool.tile([128, 1], bft)
            nc.sync.dma_start(W2_sb[:], W2[:])
            bA_sb = cpool.tile([128, W], bft)
            nc.sync.dma_start(bA_sb[:], batchA[:])
            bB_sb = cpool.tile([128, W], bft)
            nc.sync.dma_start(bB_sb[:], batchB[:])

            # h <- x  ([NPC, A] -> [128, W, A])
            nc.sync.dma_start(
                h_own[:], x_own[:].rearrange("(w p) a -> p w a", p=128))

            # ---- per-layer node-projection tables (DRAM) ----
            def project(l):
                """h_own -> P_dst table (local) + AllGathered P_src table."""
                p_dst = dpool.tile([NPC, TW], dt.float32, name=f"pdst{l}")
                p_src_b = dpool.tile([NPC, TW], dt.float32,
                                     name=f"psrcb{l}")
                p_src_all = dpool.tile([C * NPC, TW], dt.float32,
                                       name=f"psrcall{l}",
                                       addr_space=shared_as)
                with (
                    tc.tile_pool(name=f"prj{l}", bufs=3) as pp,
                    tc.tile_pool(name=f"prjp{l}", bufs=2, space="PSUM") as qq,
                ):
                    for c in range(W):
                        pt = qq.tile([A, 128], dt.float32, tag="pt")
                        nc.tensor.transpose(pt[:], h_own[:, c, :],
                                            identf_sb[:])
                        hT = pp.tile([A, 128], bft, tag="hT")
                        nc.vector.tensor_copy(hT[:], pt[:])
                        pj = qq.tile([128, PW], dt.float32, tag="pj")
                        nc.tensor.matmul(pj[:], hT[:], nW_sb[:, l, :],
                                         start=True, stop=True)
                        stg = pp.tile([128, PW], bft, tag="stg")
                        nc.vector.tensor_copy(stg[:], pj[:])
                        r = slice(c * 128, (c + 1) * 128)
                        if cfg.fp32:
                            dst_ap = p_dst[r, :]
                            src_ap = p_src_b[r, :]
                        else:
                            dst_ap = p_dst[r, :].bitcast(dt.bfloat16)
                            src_ap = p_src_b[r, :].bitcast(dt.bfloat16)
                        nc.sync.dma_start(dst_ap, stg[:, 0:TBL])
                        nc.sync.dma_start(src_ap, stg[:, TBL:PW])
                if SKIP_COLL:
                    nc.gpsimd.dma_start(out=p_src_all[0:NPC, :],
                                        in_=p_src_b[:])
                else:
                    nc.gpsimd.collective_compute(
                        "AllGather", mybir.AluOpType.bypass,
                        replica_groups=rg,
                        ins=[p_src_b[:].opt()], outs=[p_src_all[:].opt()])
                return p_dst, p_src_all

            p_dst, p_src_all = project(0)

            for l in range(L):
                # ================= edge phase =================
                abufs = 1 if cfg.fp32 else 3
                with (
                    tc.tile_pool(name=f"edg{l}", bufs=2) as ep,
                    tc.tile_pool(name=f"eact{l}", bufs=abufs) as ap_,
                    tc.tile_pool(name=f"epre{l}", bufs=2, space="PSUM") as prp,
                    tc.tile_pool(name=f"eagg{l}", bufs=1, space="PSUM") as agp,
                    tc.tile_pool(name=f"estat{l}", bufs=1, space="PSUM") as stp,
                ):
                    pstats = stp.tile([1, 2 * A], dt.float32)
                    for w in range(W):
                        idx_s = ep.tile([128, T], dt.int32, tag="ixs")
                        nc.sync.dma_start(idx_s[:], src_idx[w])
                        idx_d = ep.tile([128, T], dt.int32, tag="ixd")
                        nc.sync.dma_start(idx_d[:], dst_idx[w])
                        dw = ep.tile([128, T], bft, tag="dw")
                        nc.sync.dma_start(dw[:], dwin[w])
                        ea_t = ep.tile([EA_ROWS, T * 128], bft,
                                       tag="ea")
                        nc.sync.dma_start(ea_t[:], eaT1[:, w, :])

                        # per-chunk single-index-column gathers (the only
                        # indirect-DMA form that is exact on HW); payload is
                        # bf16 pairs packed in f32 rows, unpacked via bitcast
                        gbufs = 1 if cfg.fp32 else 2
                        Gs = ep.tile([128, T, TW], dt.float32, tag="Gs",
                                     bufs=gbufs)
                        Gd = ep.tile([128, T, TW], dt.float32, tag="Gd",
                                     bufs=gbufs)
                        for cc in range(T):
                            nc.gpsimd.indirect_dma_start(
                                out=Gs[:, cc, :], out_offset=None,
                                in_=p_src_all[:],
                                in_offset=bass.IndirectOffsetOnAxis(
                                    ap=idx_s[:, cc:cc + 1], axis=0))
                            nc.gpsimd.indirect_dma_start(
                                out=Gd[:, cc, :], out_offset=None,
                                in_=p_dst[:],
                                in_offset=bass.IndirectOffsetOnAxis(
                                    ap=idx_d[:, cc:cc + 1], axis=0))

                        pagg = agp.tile([128, A], dt.float32, tag="agg")
                        cbase = 0
                        for grp in cfg.groups:
                            pre = prp.tile([128, grp, TBL], dt.float32,
                                           tag="pre", bufs=2)
                            for j in range(grp):
                                cc = cbase + j
                                es = ea_t[:, cc * 128:(cc + 1) * 128]
                                nc.tensor.matmul(
                                    pre[:, j, 0:PREW], es, eaW_sb[:, l, :],
                                    start=True, stop=False)
                                nc.tensor.matmul(
                                    pre[:, j, 0:PREW], identb_sb[:],
                                    rhsG(Gs, cc), start=False, stop=False)
                                nc.tensor.matmul(
                                    pre[:, j, 0:PREW], identb_sb[:],
                                    rhsG(Gd, cc), start=False, stop=True)
                            # gate*softplus via Exp/Ln only (single ACT
                            # table set):  u = e^-pre_f, v = e^pre_s,
                            # sp = ln(1+v), r = 1/(1+u), msg = sp*r
                            u_t = ap_.tile([128, grp, A], dt.float32,
                                           tag="u", bufs=abufs)
                            v_t = ap_.tile([128, grp, A], dt.float32,
                                           tag="v", bufs=abufs)
                            nc.scalar.activation(u_t[:], pre[:, :, 0:A],
                                                 Act.Exp, scale=-1.0)
                            nc.scalar.activation(v_t[:], pre[:, :, A:PREW],
                                                 Act.Exp)
                            v1_t = ap_.tile([128, grp, A], dt.float32,
                                            tag="v1", bufs=abufs)
                            nc.vector.tensor_scalar(
                                out=v1_t[:], in0=v_t[:], scalar1=1.0,
                                scalar2=None, op0=Alu.add)
                            sp_t = ap_.tile([128, grp, A], dt.float32,
                                            tag="sp", bufs=abufs)
                            nc.scalar.activation(sp_t[:], v1_t[:], Act.Ln)
                            # group-batched DVE: one-hot, 1+u, 1/(1+u), msg
                            st_g = ap_.tile([128, grp, 128], bft,
                                            tag="st", bufs=abufs)
                            nc.vector.tensor_tensor(
                                out=st_g[:],
                                in0=dw[:, cbase:cbase + grp].rearrange(
                                    "p (c o) -> p c o", o=1).to_broadcast(
                                        [128, grp, 128]),
                                in1=iota_sb[:].rearrange(
                                    "p (o f) -> p o f", o=1).to_broadcast(
                                        [128, grp, 128]),
                                op=Alu.is_equal)
                            d_t = ap_.tile([128, grp, A], dt.float32,
                                           tag="d", bufs=abufs)
                            nc.vector.tensor_scalar(
                                out=d_t[:], in0=u_t[:], scalar1=1.0,
                                scalar2=None, op0=Alu.add)
                            r_t = ap_.tile([128, grp, A], dt.float32,
                                           tag="r", bufs=abufs)
                            nc.vector.reciprocal(r_t[:], d_t[:])
                            msg_g = ap_.tile([128, grp, A], bft,
                                             tag="msg", bufs=abufs)
                            nc.vector.tensor_tensor(
                                out=msg_g[:], in0=sp_t[:], in1=r_t[:],
                                op=Alu.mult)
                            for j in range(grp):
                                cc = cbase + j
                                nc.tensor.matmul(
                                    pagg[:], st_g[:, j, :], msg_g[:, j, :],
                                    start=(cc == 0), stop=(cc == T - 1))
                            cbase += grp
                        # window epilogue: stash agg, BN partial stats
                        # (single accumulation group: rhs = [agg | agg^2])
                        nc.vector.tensor_copy(agg_own[:, w, :], pagg[:])
                        sqa = ap_.tile([128, 2 * A], dt.float32, tag="sqa",
                                       bufs=2)
                        nc.vector.tensor_copy(sqa[:, 0:A], agg_own[:, w, :])
                        nc.vector.tensor_tensor(
                            out=sqa[:, A:2 * A], in0=agg_own[:, w, :],
                            in1=agg_own[:, w, :], op=Alu.mult)
                        nc.tensor.matmul(pstats[0:1, :], onesc_sb[:],
                                         sqa[:],
                                         start=(w == 0), stop=(w == W - 1))
                    stats_sb = ap_.tile([1, 2 * A], dt.float32, bufs=1)
                    nc.vector.tensor_copy(stats_sb[:], pstats[:])
                    st_in = dpool.tile([1, 2 * A], dt.float32,
                                       name=f"stin{l}")
                    st_out = dpool.tile([1, 2 * A], dt.float32,
                                        name=f"stout{l}", addr_space=shared_as)
                    nc.sync.dma_start(st_in[:], stats_sb[:])

                if SKIP_COLL:
                    nc.gpsimd.dma_start(out=st_out[:], in_=st_in[:])
                else:
                    nc.gpsimd.collective_compute(
                        "AllReduce", Alu.add, replica_groups=rg,
                        ins=[st_in[:].opt()], outs=[st_out[:].opt()])

                # ================= BN + update =================
                with (
                    tc.tile_pool(name=f"bn{l}", bufs=1) as bp,
                    tc.tile_pool(name=f"bnp{l}", bufs=1, space="PSUM") as bq,
                    tc.tile_pool(name=f"upd{l}", bufs=2) as up,
                ):
                    sg = bp.tile([1, 2 * A], dt.float32)
                    nc.sync.dma_start(sg[:], st_out[:])
                    mean = bp.tile([1, A], dt.float32)
                    nc.vector.tensor_scalar(
                        out=mean[:], in0=sg[0:1, 0:A],
                        scalar1=1.0 / cfg.n_real_nodes, scalar2=None,
                        op0=Alu.mult)
                    var = bp.tile([1, A], dt.float32)
                    nc.vector.tensor_scalar(
                        out=var[:], in0=sg[0:1, A:2 * A],
                        scalar1=1.0 / cfg.n_real_nodes, scalar2=None,
                        op0=Alu.mult)
                    m2 = bp.tile([1, A], dt.float32)
                    nc.vector.tensor_tensor(out=m2[:], in0=mean[:],
                                            in1=mean[:], op=Alu.mult)
                    nc.vector.tensor_tensor(out=var[:], in0=var[:],
                                            in1=m2[:], op=Alu.subtract)
                    nc.vector.tensor_scalar(out=var[:], in0=var[:],
                                            scalar1=BN_EPS, scalar2=None,
                                            op0=Alu.add)
                    # rstd = (var+eps)^-0.5 = Exp(-0.5 * Ln(var+eps))
                    lv = bp.tile([1, A], dt.float32)
                    nc.scalar.activation(lv[:], var[:], Act.Ln)
                    rstd = bp.tile([1, A], dt.float32)
                    nc.scalar.activation(rstd[:], lv[:], Act.Exp, scale=-0.5)
                    scl = bp.tile([1, A], dt.float32)
                    nc.vector.tensor_tensor(out=scl[:], in0=rstd[:],
                                            in1=gb_sb[0:1, 2 * l, :],
                                            op=Alu.mult)
                    shf = bp.tile([1, A], dt.float32)
                    nc.vector.tensor_tensor(out=shf[:], in0=mean[:],
                                            in1=scl[:], op=Alu.mult)
                    nc.vector.tensor_tensor(out=shf[:],
                                            in0=gb_sb[0:1, 2 * l + 1, :],
                                            in1=shf[:], op=Alu.subtract)
                    # broadcast rows -> [128, A] tiles (K=1 matmuls)
                    pb = bq.tile([128, 2 * A], dt.float32)
                    nc.tensor.matmul(pb[:, 0:A], onesr_sb[:], scl[:],
                                     start=True, stop=True)
                    nc.tensor.matmul(pb[:, A:2 * A], onesr_sb[:], shf[:],
                                     start=True, stop=True)
                    sc_bc = bp.tile([128, A], dt.float32)
                    nc.vector.tensor_copy(sc_bc[:], pb[:, 0:A])
                    sh_bc = bp.tile([128, A], dt.float32)
                    nc.vector.tensor_copy(sh_bc[:], pb[:, A:2 * A])

                    # h = lrelu(agg*scale + shift + h)
                    UB = 7
                    for c0 in range(0, W, UB):
                        k = min(UB, W - c0)
                        tmp = up.tile([128, UB, A], dt.float32, tag="tmp")
                        for j in range(k):
                            c = c0 + j
                            nc.vector.tensor_tensor(
                                out=tmp[:, j, :], in0=agg_own[:, c, :],
                                in1=sc_bc[:], op=Alu.mult)
                            nc.vector.tensor_tensor(
                                out=tmp[:, j, :], in0=tmp[:, j, :],
                                in1=sh_bc[:], op=Alu.add)
                            nc.vector.tensor_tensor(
                                out=tmp[:, j, :], in0=tmp[:, j, :],
                                in1=h_own[:, c, :], op=Alu.add)
                        # leaky relu: h = max(0.01*t, t) in one fused DVE op
                        nc.vector.scalar_tensor_tensor(
                            out=h_own[:, c0:c0 + k, :], in0=tmp[:, 0:k, :],
                            scalar=NEG_SLOPE, in1=tmp[:, 0:k, :],
                            op0=Alu.mult, op1=Alu.max)

                if l + 1 < L:
                    p_dst, p_src_all = project(l + 1)

            # ================= final MLP + pooling =================
            with (
                tc.tile_pool(name="fin", bufs=3) as fp,
                tc.tile_pool(name="finp", bufs=2, space="PSUM") as fq,
                tc.tile_pool(name="fing", bufs=1, space="PSUM") as gq,
            ):
                pgA = gq.tile([128, 1], dt.float32, name="pgA")
                pgB = gq.tile([128, 1], dt.float32, name="pgB")
                pgs = [pgA, pgB]
                for c in range(W):
                    pt = fq.tile([A, 128], dt.float32, tag="pt")
                    nc.tensor.transpose(pt[:], h_own[:, c, :], identf_sb[:])
                    hT = fp.tile([A, 128], bft, tag="hT")
                    nc.vector.tensor_copy(hT[:], pt[:])
                    pz = fq.tile([128, 128], dt.float32, tag="pz")
                    nc.tensor.matmul(pz[:], W1_sb[:], hT[:],
                                     start=True, stop=True)
                    zb = fp.tile([128, 128], dt.float32, tag="zb")
                    nc.vector.tensor_scalar(
                        out=zb[:], in0=pz[:], scalar1=b1_sb[:, 0:1],
                        scalar2=None, op0=Alu.add)
                    z1 = fp.tile([128, 128], bft, tag="z1")
                    nc.vector.scalar_tensor_tensor(
                        out=z1[:], in0=zb[:], scalar=NEG_SLOPE, in1=zb[:],
                        op0=Alu.mult, op1=Alu.max)
                    po = fq.tile([128, 1], dt.float32, tag="po")
                    nc.tensor.matmul(po[:], z1[:], W2_sb[:],
                                     start=True, stop=True)
                    ov = fp.tile([128, 1], bft, tag="ov")
                    nc.vector.tensor_copy(ov[:], po[:])
                    for gh, bsb in enumerate([bA_sb, bB_sb][:NG2]):
                        oh = fp.tile([128, 128], bft, tag="oh",
                                     bufs=3)
                        nc.vector.tensor_tensor(
                            out=oh[:],
                            in0=bsb[:, c:c + 1].to_broadcast([128, 128]),
                            in1=iota_sb[:], op=Alu.is_equal)
                        nc.tensor.matmul(pgs[gh][:], oh[:], ov[:],
                                         start=(c == 0), stop=(c == W - 1))
                gs = fp.tile([128, NG2], dt.float32)
                for gh in range(NG2):
                    nc.vector.tensor_copy(gs[:, gh:gh + 1], pgs[gh][:])
                nc.sync.dma_start(out_part[:], gs[:])

    nc.compile()
    return nc


# --------------------------------------------------------------------------
# entry point
# --------------------------------------------------------------------------

def postprocess(results, cfg, post):
    NG2 = 2 if cfg.n_graphs > 128 else 1
    total = np.zeros((128 * NG2,), np.float64)
    for r in results:
        o = np.asarray(r["out_partial"], np.float32)  # [128, NG2]
        total += o.transpose(1, 0).reshape(-1).astype(np.float64)
    total = total[:cfg.n_graphs]
    out = total / np.maximum(post["counts"], 1.0) + post["b2"]
    return out.reshape(-1, 1).astype(np.float32)


def kernel(**inputs):
    _ensure_path()
    from concourse.bass_utils import run_bass_kernel_spmd

    cfg, in_maps, post = preprocess(inputs, N_CORES)
    nc = build_program(cfg)
    res = run_bass_kernel_spmd(nc, in_maps, core_ids=list(range(N_CORES)))
    return postprocess(res.results, cfg, post)
